# revision 5
# baseline (speedup 1.0000x reference)
"""KD loss (teacher softmax x student log-softmax, masked mean) on 8 TRN2 cores.

Sharding: data-parallel over the 4096 tokens -- 512 tokens per core.
Each core streams its (512, 32000) slices of student/teacher logits once
and produces per-token stats; the host does the final tiny reduction.

Per token t over vocab i:
    Z_t  = sum_i exp(teacher_i)
    Z_x  = sum_i exp(student_i)
    cross = sum_i exp(teacher_i) * student_i
    x_t  = cross / Z_t - ln(Z_x)           # = sum_i p_i * logsoftmax(x)_i
    loss = -sum_t x_t * mask_t / sum_t mask_t

No max-subtraction: inputs are standard normal (|logit| < ~6), so exp is
safe in fp32 and sums (~5e4) are well within range.
"""

import numpy as np

_B, _S, _V = 2, 2048, 32000
_N = _B * _S                      # 4096 tokens
_NCORES = 8
_TOK = _N // _NCORES              # 512 tokens per core
_P = 128                          # SBUF partitions
_NTILES = _TOK // _P              # 4 partition-tiles per core
_F = 3200                         # vocab chunk (free-dim) per DMA/compute tile
_NCHUNK = _V // _F                # 10 chunks

_cache = {}


def _build():
    import concourse.bacc as bacc
    import concourse.mybir as mybir
    import concourse.tile as tile

    f32 = mybir.dt.float32
    AF = mybir.ActivationFunctionType
    ALU = mybir.AluOpType
    AX = mybir.AxisListType

    nc = bacc.Bacc()
    student = nc.dram_tensor("student", [_TOK, _V], f32, kind="ExternalInput")
    teacher = nc.dram_tensor("teacher", [_TOK, _V], f32, kind="ExternalInput")
    out = nc.dram_tensor("out", [_P, _NTILES], f32, kind="ExternalOutput")

    with tile.TileContext(nc) as tc:
        with (
            tc.tile_pool(name="io", bufs=3) as io,
            tc.tile_pool(name="scratch", bufs=2) as scratch,
            tc.tile_pool(name="stats", bufs=1) as stats,
            tc.tile_pool(name="fin", bufs=4) as fin,
        ):
            ncols = _NTILES * _NCHUNK
            zt_cols = stats.tile([_P, ncols], f32)   # per-chunk sum exp(teacher)
            zx_cols = stats.tile([_P, ncols], f32)   # per-chunk sum exp(student)
            cr_cols = stats.tile([_P, ncols], f32)   # per-chunk sum exp(t)*x
            x_out = stats.tile([_P, _NTILES], f32)

            for it in range(_NTILES):
                rows = slice(it * _P, (it + 1) * _P)
                for j in range(_NCHUNK):
                    cols = slice(j * _F, (j + 1) * _F)
                    k = it * _NCHUNK + j

                    tT = io.tile([_P, _F], f32)
                    nc.sync.dma_start(out=tT[:, :], in_=teacher[rows, cols])
                    tX = io.tile([_P, _F], f32)
                    nc.sync.dma_start(out=tX[:, :], in_=student[rows, cols])

                    # exp(teacher) -> eT, and Z_t partial in one ACT op
                    eT = scratch.tile([_P, _F], f32)
                    nc.scalar.activation(
                        eT[:, :], tT[:, :], AF.Exp,
                        accum_out=zt_cols[:, k : k + 1],
                    )
                    # exp(student) only needed for its sum
                    eX = scratch.tile([_P, _F], f32)
                    nc.scalar.activation(
                        eX[:, :], tX[:, :], AF.Exp,
                        accum_out=zx_cols[:, k : k + 1],
                    )
                    # cross partial: one fused DVE multiply+accumulate
                    # out = (eT * 1.0) * tX, accum_out = sum(out)
                    # The tiny copy first absorbs the DMA-completion wait on
                    # tX onto a cheap DVE op: the STT ISA struct has too few
                    # sync-wait slots for two cross-engine waits (walrus
                    # "Too many sync wait commands" otherwise).
                    sink = scratch.tile([_P, 1], f32)
                    nc.vector.tensor_copy(sink[:, :], tX[:, 0:1])
                    prod = scratch.tile([_P, _F], f32)
                    nc.vector.scalar_tensor_tensor(
                        out=prod[:, :],
                        in0=eT[:, :],
                        scalar=1.0,
                        in1=tX[:, :],
                        op0=ALU.mult,
                        op1=ALU.mult,
                        accum_out=cr_cols[:, k : k + 1],
                    )

            for it in range(_NTILES):
                s = slice(it * _NCHUNK, (it + 1) * _NCHUNK)
                zt = fin.tile([_P, 1], f32)
                nc.vector.tensor_reduce(zt[:, :], zt_cols[:, s], axis=AX.X, op=ALU.add)
                zx = fin.tile([_P, 1], f32)
                nc.vector.tensor_reduce(zx[:, :], zx_cols[:, s], axis=AX.X, op=ALU.add)
                cr = fin.tile([_P, 1], f32)
                nc.vector.tensor_reduce(cr[:, :], cr_cols[:, s], axis=AX.X, op=ALU.add)
                rec = fin.tile([_P, 1], f32)
                nc.vector.reciprocal(rec[:, :], zt[:, :])
                term = fin.tile([_P, 1], f32)
                nc.vector.tensor_mul(term[:, :], cr[:, :], rec[:, :])
                lnzx = fin.tile([_P, 1], f32)
                nc.scalar.activation(lnzx[:, :], zx[:, :], AF.Ln)
                nc.vector.tensor_sub(x_out[:, it : it + 1], term[:, :], lnzx[:, :])

            nc.sync.dma_start(out=out[:, :], in_=x_out[:, :])

    nc.finalize()
    return nc


def _run(student_2d, teacher_2d, trace=False):
    """student_2d/teacher_2d: (4096, 32000) f32 C-contiguous.
    Returns (x_tokens[8,128,NTILES], BassKernelResults)."""
    from concourse.bass_utils import run_bass_kernel_spmd

    if "nc" not in _cache:
        _cache["nc"] = _build()
    nc = _cache["nc"]

    in_maps = []
    for c in range(_NCORES):
        rows = slice(c * _TOK, (c + 1) * _TOK)
        in_maps.append(
            {
                "student": np.ascontiguousarray(student_2d[rows]),
                "teacher": np.ascontiguousarray(teacher_2d[rows]),
            }
        )
    res = run_bass_kernel_spmd(
        nc, in_maps, core_ids=list(range(_NCORES)), trace=trace
    )
    x = np.stack([r["out"] for r in res.results])  # [8, 128, NTILES]
    return x, res


def kernel(logits, teacher_logits, labels):
    lg = np.ascontiguousarray(np.asarray(logits, dtype=np.float32).reshape(_N, _V))
    tg = np.ascontiguousarray(
        np.asarray(teacher_logits, dtype=np.float32).reshape(_N, _V)
    )
    x, _ = _run(lg, tg, trace=False)
    # x[c, p, i] holds token c*512 + i*128 + p
    xt = x.transpose(0, 2, 1).reshape(_N)
    lab = np.asarray(labels).reshape(_N)
    mask = lab != -100
    loss = -(xt[mask].astype(np.float64).sum()) / max(int(mask.sum()), 1)
    return np.asarray(loss, dtype=np.float32)


# revision 6
# speedup vs baseline: 1.2528x; 1.2528x over previous
"""KD loss (teacher softmax x student log-softmax, masked mean) on 8 TRN2 cores.

Sharding: data-parallel over the 4096 tokens -- 512 tokens per core.
Each core streams its (512, 32000) slices of student/teacher logits once
and produces per-token stats; the host does the final tiny reduction.

Per token t over vocab i:
    Z_t  = sum_i exp(teacher_i)
    Z_x  = sum_i exp(student_i)
    cross = sum_i exp(teacher_i) * student_i
    x_t  = cross / Z_t - ln(Z_x)           # = sum_i p_i * logsoftmax(x)_i
    loss = -sum_t x_t * mask_t / sum_t mask_t

No max-subtraction: inputs are standard normal (|logit| < ~6), so exp is
safe in fp32 and sums (~5e4) are well within range.
"""

import numpy as np

_B, _S, _V = 2, 2048, 32000
_N = _B * _S                      # 4096 tokens
_NCORES = 8
_TOK = _N // _NCORES              # 512 tokens per core
_P = 128                          # SBUF partitions
_NTILES = _TOK // _P              # 4 partition-tiles per core
_F = 6400                         # vocab chunk (free-dim) per DMA/compute tile
_NCHUNK = _V // _F                # 5 chunks

_cache = {}


def _build():
    import concourse.bacc as bacc
    import concourse.mybir as mybir
    import concourse.tile as tile

    f32 = mybir.dt.float32
    AF = mybir.ActivationFunctionType
    ALU = mybir.AluOpType
    AX = mybir.AxisListType

    nc = bacc.Bacc()
    student = nc.dram_tensor("student", [_TOK, _V], f32, kind="ExternalInput")
    teacher = nc.dram_tensor("teacher", [_TOK, _V], f32, kind="ExternalInput")
    out = nc.dram_tensor("out", [_P, _NTILES], f32, kind="ExternalOutput")

    with tile.TileContext(nc) as tc:
        with (
            tc.tile_pool(name="io", bufs=2) as io,
            tc.tile_pool(name="scratch", bufs=2) as scratch,
            tc.tile_pool(name="stats", bufs=1) as stats,
            tc.tile_pool(name="fin", bufs=1) as fin,
        ):
            ncols = _NTILES * _NCHUNK
            zt_cols = stats.tile([_P, ncols], f32)   # per-chunk sum exp(teacher)
            zx_cols = stats.tile([_P, ncols], f32)   # per-chunk sum exp(student)
            cr_cols = stats.tile([_P, ncols], f32)   # per-chunk sum exp(t)*x
            x_out = stats.tile([_P, _NTILES], f32)

            for it in range(_NTILES):
                rows = slice(it * _P, (it + 1) * _P)
                for j in range(_NCHUNK):
                    cols = slice(j * _F, (j + 1) * _F)
                    k = it * _NCHUNK + j

                    tT = io.tile([_P, _F], f32)
                    nc.sync.dma_start(out=tT[:, :], in_=teacher[rows, cols])
                    tX = io.tile([_P, _F], f32)
                    nc.sync.dma_start(out=tX[:, :], in_=student[rows, cols])

                    # exp(teacher) -> eT, and Z_t partial in one ACT op
                    eT = scratch.tile([_P, _F], f32)
                    nc.scalar.activation(
                        eT[:, :], tT[:, :], AF.Exp,
                        accum_out=zt_cols[:, k : k + 1],
                    )
                    # exp(student): only its free-dim sum is needed, so the
                    # full output is discarded through a stride-0 AP
                    xsink = scratch.tile([_P, 1], f32)
                    nc.scalar.activation(
                        xsink.broadcast_to((_P, _F)), tX[:, :], AF.Exp,
                        accum_out=zx_cols[:, k : k + 1],
                    )
                    # cross partial: one fused DVE multiply+accumulate
                    # out = (eT * 1.0) * tX, accum_out = sum(out)
                    # The tiny copy first absorbs the DMA-completion wait on
                    # tX onto a cheap DVE op: the STT ISA struct has too few
                    # sync-wait slots for two cross-engine waits (walrus
                    # "Too many sync wait commands" otherwise).
                    sink = scratch.tile([_P, 1], f32)
                    nc.vector.tensor_copy(sink[:, :], tX[:, 0:1])
                    psink = scratch.tile([_P, 1], f32)
                    nc.vector.scalar_tensor_tensor(
                        out=psink.broadcast_to((_P, _F)),
                        in0=eT[:, :],
                        scalar=1.0,
                        in1=tX[:, :],
                        op0=ALU.mult,
                        op1=ALU.mult,
                        accum_out=cr_cols[:, k : k + 1],
                    )

            # Final per-token math, batched over all 4 partition-tiles:
            # view [P, ntiles*nchunk] stats as [P, ntiles, nchunk] and
            # reduce the chunk axis.
            zt4 = fin.tile([_P, _NTILES], f32)
            nc.vector.tensor_reduce(
                zt4[:, :],
                zt_cols[:, :].rearrange("p (a b) -> p a b", a=_NTILES),
                axis=AX.X, op=ALU.add,
            )
            zx4 = fin.tile([_P, _NTILES], f32)
            nc.vector.tensor_reduce(
                zx4[:, :],
                zx_cols[:, :].rearrange("p (a b) -> p a b", a=_NTILES),
                axis=AX.X, op=ALU.add,
            )
            cr4 = fin.tile([_P, _NTILES], f32)
            nc.vector.tensor_reduce(
                cr4[:, :],
                cr_cols[:, :].rearrange("p (a b) -> p a b", a=_NTILES),
                axis=AX.X, op=ALU.add,
            )
            rec4 = fin.tile([_P, _NTILES], f32)
            nc.vector.reciprocal(rec4[:, :], zt4[:, :])
            term4 = fin.tile([_P, _NTILES], f32)
            nc.vector.tensor_mul(term4[:, :], cr4[:, :], rec4[:, :])
            ln4 = fin.tile([_P, _NTILES], f32)
            nc.scalar.activation(ln4[:, :], zx4[:, :], AF.Ln)
            nc.vector.tensor_sub(x_out[:, :], term4[:, :], ln4[:, :])

            nc.sync.dma_start(out=out[:, :], in_=x_out[:, :])

    nc.finalize()
    return nc


def _run(student_2d, teacher_2d, trace=False):
    """student_2d/teacher_2d: (4096, 32000) f32 C-contiguous.
    Returns (x_tokens[8,128,NTILES], BassKernelResults)."""
    from concourse.bass_utils import run_bass_kernel_spmd

    if "nc" not in _cache:
        _cache["nc"] = _build()
    nc = _cache["nc"]

    in_maps = []
    for c in range(_NCORES):
        rows = slice(c * _TOK, (c + 1) * _TOK)
        in_maps.append(
            {
                "student": np.ascontiguousarray(student_2d[rows]),
                "teacher": np.ascontiguousarray(teacher_2d[rows]),
            }
        )
    res = run_bass_kernel_spmd(
        nc, in_maps, core_ids=list(range(_NCORES)), trace=trace
    )
    x = np.stack([r["out"] for r in res.results])  # [8, 128, NTILES]
    return x, res


def kernel(logits, teacher_logits, labels):
    lg = np.ascontiguousarray(np.asarray(logits, dtype=np.float32).reshape(_N, _V))
    tg = np.ascontiguousarray(
        np.asarray(teacher_logits, dtype=np.float32).reshape(_N, _V)
    )
    x, _ = _run(lg, tg, trace=False)
    # x[c, p, i] holds token c*512 + i*128 + p
    xt = x.transpose(0, 2, 1).reshape(_N)
    lab = np.asarray(labels).reshape(_N)
    mask = lab != -100
    loss = -(xt[mask].astype(np.float64).sum()) / max(int(mask.sum()), 1)
    return np.asarray(loss, dtype=np.float32)
